# revision 9
# baseline (speedup 1.0000x reference)
"""Causal self-attention (B=2, T=2048, L=1024, H=16) on 8 TRN2 NeuronCores.

Sharding: tensor-parallel over heads (4 heads/core) x data-parallel over batch
(cores 0-3 -> batch 0, cores 4-7 -> batch 1). Each core computes its heads'
attention plus the partial output projection; the host sums the 4 partials
per batch.

Per-core pipeline (all matmul operands bf16):
  stage A: Q^T/K^T [256, 2048] (head dims on partitions), V [2048, 4*64].
  stage B per 512-wide q-block: score chunks S^T [128 k, <=512 q] computed as
    row-tiled matmul PAIRS (two heads' 64-wide contractions run concurrently
    in the PE array via tile_position rows 0-63 / 64-127). Chunks are scored
    in [128, 1024] double-bank PSUM tiles so exp runs over two chunks per
    activation call. exp (no max subtraction; scores ~N(0, 0.25)) -> bf16.
    PV accumulation as col-tiled matmul PAIRS (head A -> PSUM partitions
    0-63, head B -> 64-127). Softmax denominators accumulate in a shared
    PSUM bank via ones-weight matmuls col-tiled 4 ways (partition strips
    0/32/64/96).
  normalize: reciprocal of the four denominator rows, one broadcast matmul
    per head-pair expands 1/den to all 64 dv partitions, vector multiply.
  stage C: out[t, n] = y^T.T @ W_proj_slice accumulated in PSUM, DMA'd
    straight from PSUM to DRAM per [128, 512] tile.
"""

import sys

for _p in ("/opt/trn_rl_repo",):
    if _p not in sys.path:
        sys.path.insert(0, _p)

import numpy as np

import concourse.bass as bass
import concourse.mybir as mybir
import concourse.tile as tile

F32 = mybir.dt.float32
BF16 = mybir.dt.bfloat16
EXP = mybir.ActivationFunctionType.Exp

B, T, L = 2, 2048, 1024
H = 16
DH = 64                      # head dim
HPC = 4                      # heads per core
HG = HPC * DH                # 256 cols per core per q/k/v
N_CORES = 8
KC = T // 128                # 16 k-chunks
NQB = T // 512               # 4 q-blocks
SCALE = 1.0 / np.sqrt(np.float32(L))  # rsqrt(L) per reference


def build_nc(iters=1):
    nc = bass.Bass("TRN2", target_bir_lowering=False, debug=False)

    xT = nc.dram_tensor("xT", [L, T], BF16, kind="ExternalInput").ap()
    wa = nc.dram_tensor("wa", [L, 3 * HG], BF16, kind="ExternalInput").ap()
    wp = nc.dram_tensor("wp", [HG, L], BF16, kind="ExternalInput").ap()
    tm = nc.dram_tensor("trimaskb", [128, 128], F32, kind="ExternalInput").ap()
    idn = nc.dram_tensor("ident", [128, 128], F32, kind="ExternalInput").ap()
    out = nc.dram_tensor("out", [T, L], F32, kind="ExternalOutput").ap()

    with tile.TileContext(nc) as tc:
        with (
            tc.tile_pool(name="consts", bufs=1) as consts,
            tc.tile_pool(name="xp", bufs=8) as xp,
            tc.tile_pool(name="wap", bufs=8) as wap,
            tc.tile_pool(name="wpp", bufs=2) as wpp,
            tc.tile_pool(name="qk", bufs=2) as qk,
            tc.tile_pool(name="vp", bufs=16) as vp,
            tc.tile_pool(name="ytp", bufs=2) as ytp,
            tc.tile_pool(name="ptp", bufs=6) as ptp,
            tc.tile_pool(name="bcp", bufs=2) as bcp,
            tc.tile_pool(name="outp", bufs=2) as outp,
            tc.tile_pool(name="pss", bufs=4, space="PSUM") as pss,   # scores: 4 banks
            tc.tile_pool(name="pop", bufs=1, space="PSUM") as pop,   # 1 bank
            tc.tile_pool(name="psdp", bufs=1, space="PSUM") as psdp, # 1 bank
            tc.tile_pool(name="p2", bufs=2, space="PSUM") as p2,     # bp/psmm/psc: 2 banks
        ):
            # ---- constants ----
            tm_sb = consts.tile([128, 128], BF16)
            nc.gpsimd.dma_start(out=tm_sb[:], in_=tm[:])
            id_sb = consts.tile([128, 128], BF16)
            nc.gpsimd.dma_start(out=id_sb[:], in_=idn[:])
            ones_sb = consts.tile([128, 128], BF16)
            nc.vector.memset(ones_sb[:], 1.0)
            # broadcast selectors: sel[pr] is [97, 128]; row 32*(2pr+hh) has 1s
            # in cols hh*64:(hh+1)*64. rec2 rows outside {0,32,64,96} stay 0.
            sel = []
            for pr in range(2):
                s = consts.tile([97, 128], BF16, name=f"sel{pr}")
                nc.vector.memset(s[:], 0.0)
                nc.vector.memset(s[64 * pr:64 * pr + 1, 0:64], 1.0)
                nc.vector.memset(s[64 * pr + 32:64 * pr + 33, 64:128], 1.0)
                sel.append(s)
            rec2 = consts.tile([128, 512], BF16, name="rec2")
            nc.vector.memset(rec2[:], 0.0)

            for _it in range(iters):
                _body(nc, xT, wa, wp, out, tm_sb, id_sb, ones_sb, sel, rec2,
                      xp, wap, wpp, qk, vp, ytp, ptp, bcp, outp, pss, pop, psdp, p2, _it)

    import os as _os
    if not _os.environ.get("KERNEL_SKIP_WAITFIX"):
        _fix_matmul_waits(nc)
    return nc


def _body(nc, xT, wa, wp, out, tm_sb, id_sb, ones_sb, sel, rec2,
          xp, wap, wpp, qk, vp, ytp, ptp, bcp, outp, pss, pop, psdp, p2, it):
    # ---- input loads ----
    xt_sb = []
    wa_sb = []
    for kc in range(8):
        xt = xp.tile([128, T], BF16, tag="xt")
        nc.sync.dma_start(out=xt[:], in_=xT[kc * 128:(kc + 1) * 128, :])
        xt_sb.append(xt)
        wat = wap.tile([128, 3 * HG], BF16, tag="wa")
        nc.sync.dma_start(out=wat[:], in_=wa[kc * 128:(kc + 1) * 128, :])
        wa_sb.append(wat)
    wp_sb = []
    for i in range(2):
        wpt = wpp.tile([128, L], BF16, tag="wp")
        nc.sync.dma_start(out=wpt[:], in_=wp[i * 128:(i + 1) * 128, :])
        wp_sb.append(wpt)

    # ---- stage A: Q^T, K^T [256, T]; V [T, 4*64] ----
    qt = [qk.tile([128, T], BF16, tag="qt", name=f"qt{m}_{it}") for m in range(2)]
    kt = [qk.tile([128, T], BF16, tag="kt", name=f"kt{m}_{it}") for m in range(2)]
    for nb in range(NQB):
        for dst, coff, eng in ((qt, 0, "s"), (kt, HG, "v")):
            for m in range(2):
                ps = p2.tile([128, 512], F32, tag="p2")
                for kc in range(8):
                    nc.tensor.matmul(
                        ps[:],
                        wa_sb[kc][:, coff + m * 128:coff + (m + 1) * 128],
                        xt_sb[kc][:, nb * 512:(nb + 1) * 512],
                        start=(kc == 0),
                        stop=(kc == 7),
                    )
                d = dst[m][:, nb * 512:(nb + 1) * 512]
                if eng == "s":
                    nc.scalar.copy(d, ps[:])
                else:
                    nc.vector.tensor_copy(d, ps[:])

    va_sb = []
    for tt in range(KC):
        ps = p2.tile([128, 512], F32, tag="p2")
        for kc in range(8):
            nc.tensor.matmul(
                ps[:, 0:HG],
                xt_sb[kc][:, tt * 128:(tt + 1) * 128],
                wa_sb[kc][:, 2 * HG:3 * HG],
                start=(kc == 0),
                stop=(kc == 7),
            )
        va = vp.tile([128, HG], BF16, tag="va")
        nc.scalar.copy(va[:], ps[:, 0:HG])
        va_sb.append(va)

    # ---- stage B + C per q-block ----
    yt = [ytp.tile([128, T], BF16, tag="yt", name=f"yt{m}_{it}") for m in range(2)]
    for qb in range(NQB):
        nkc = 4 * qb + 4
        psd = psdp.tile([128, 512], F32, tag="psd", name=f"psd{qb}_{it}")
        for pr in range(2):
            po = pop.tile([128, 512], F32, tag="po", name=f"po{qb}_{pr}_{it}")

            def emit_av(kc, ncols, ptA, ptB):
                j = kc - 4 * qb
                a0 = 128 * j if j >= 0 else 0
                first, last = kc == 0, kc == nkc - 1
                mid = not (first or last)
                # col-tiled pair: head A -> po rows 0-63, head B -> 64-127
                nc.tensor.matmul(
                    po[0:64, a0:512],
                    va_sb[kc][:, pr * 128:pr * 128 + 64],
                    ptA[:, 0:ncols],
                    start=first, stop=last, skip_group_check=mid,
                )
                nc.tensor.matmul(
                    po[64:128, a0:512],
                    va_sb[kc][:, pr * 128 + 64:pr * 128 + 128],
                    ptB[:, 0:ncols],
                    start=first, stop=last, skip_group_check=mid,
                )
                # denominators: ones-weight matmuls col-tiled to strips
                # 32*(2pr) and 32*(2pr+1) of the shared psd bank
                for hh, pt in ((0, ptA), (1, ptB)):
                    s = 2 * pr + hh
                    nc.tensor.matmul(
                        psd[32 * s:32 * s + 1, a0:512],
                        ones_sb[:, 0:1],
                        pt[:, 0:ncols],
                        start=first, stop=last, skip_group_check=mid,
                        tile_position=(0, 32 * s),
                    )

            prev = None
            for kc in range(nkc):
                j = kc - 4 * qb
                q0 = qb * 512 + (128 * j if j >= 0 else 0)
                ncols = 512 - (128 * j if j >= 0 else 0)
                psA = pss.tile([128, 512], F32, tag="ps2")
                psB = pss.tile([128, 512], F32, tag="ps2")
                for hh, ps2 in ((0, psA), (1, psB)):
                    # two heads' 64-contract scores run concurrently via
                    # row tiling (partitions 0-63 / 64-127)
                    nc.tensor.matmul(
                        ps2[:, 0:ncols],
                        kt[pr][hh * 64:(hh + 1) * 64, kc * 128:(kc + 1) * 128],
                        qt[pr][hh * 64:(hh + 1) * 64, q0:q0 + ncols],
                        start=True,
                        stop=(j < 0),
                    )
                    if j >= 0:
                        # -3e30 above the diagonal: psum += I.T @ tri_bias
                        nc.tensor.matmul(
                            ps2[:, 0:128],
                            id_sb[:],
                            tm_sb[:],
                            start=False,
                            stop=True,
                        )
                if prev is not None:
                    emit_av(*prev)
                ptA = ptp.tile([128, 512], BF16, tag="pt")
                ptB = ptp.tile([128, 512], BF16, tag="pt")
                nc.scalar.activation(ptA[:, 0:ncols], psA[:, 0:ncols], EXP,
                                     scale=float(SCALE))
                nc.scalar.activation(ptB[:, 0:ncols], psB[:, 0:ncols], EXP,
                                     scale=float(SCALE))
                prev = (kc, ncols, ptA, ptB)
            emit_av(*prev)

            # ---- normalize this head-pair ----
            for hh in range(2):
                s = 2 * pr + hh
                with nc.allow_low_precision(reason="softmax denom recip to bf16"):
                    nc.vector.reciprocal(rec2[32 * s:32 * s + 1, :],
                                         psd[32 * s:32 * s + 1, :])
            bp = p2.tile([128, 512], F32, tag="p2")
            nc.tensor.matmul(bp[:], sel[pr][0:97, :], rec2[0:97, :],
                             start=True, stop=True)
            bs = bcp.tile([128, 512], F32, tag="bs")
            nc.vector.tensor_copy(bs[:], bp[:])
            nc.vector.tensor_mul(
                yt[pr][:, qb * 512:(qb + 1) * 512], po[:], bs[:])

        # ---- stage C for this q-block's 4 t-tiles ----
        for tt in range(4 * qb, 4 * qb + 4):
            osb = outp.tile([128, L], F32, tag="osb")
            for nn in range(2):
                ps = p2.tile([128, 512], F32, tag="p2")
                for pr in range(2):
                    nc.tensor.matmul(
                        ps[:],
                        yt[pr][:, tt * 128:(tt + 1) * 128],
                        wp_sb[pr][:, nn * 512:(nn + 1) * 512],
                        start=(pr == 0),
                        stop=(pr == 1),
                    )
                nc.vector.tensor_copy(osb[:, nn * 512:(nn + 1) * 512], ps[:])
            nc.sync.dma_start(out=out[tt * 128:(tt + 1) * 128, :], in_=osb[:])


def _fix_matmul_waits(nc):
    """walrus caps sync-wait commands at one per hardware instruction.
    Tile can emit more. Two safe fixes, applied in order:
    1. drop waits on the instruction's own engine semaphore that are already
       guaranteed by in-order retirement of earlier same-stream instructions;
    2. for any instruction still holding >1 wait, insert same-engine NoOps
       immediately before it, each carrying one excess wait (the waits still
       all execute before the instruction dispatches).
    """
    import bass_rust
    import concourse.mybir as mybir

    SKIP = (mybir.InstEventSemaphore, mybir.InstCall,
            mybir.InstUnconditionalBranch)
    nop_id = [0]

    for f in nc.m.functions:
        for blk in f.blocks:
            insts = list(blk.instructions)
            out = []
            changed = False
            for i, inst in enumerate(insts):
                si = inst.sync_info
                eng = getattr(inst, "engine", None)
                if si is None or eng is None or isinstance(inst, SKIP):
                    out.append(inst)
                    continue
                waits = list(si.on_wait)
                kept = waits
                if len(kept) > 1:
                    for w in kept[:-1]:
                        nop = mybir.InstNoOp(name=f"I-waitnop-{nop_id[0]}")
                        nop_id[0] += 1
                        nop.engine = eng
                        nop.sync_info = bass_rust.SyncInfo(on_wait=[w], on_update=[])
                        out.append(nop)
                    kept = kept[-1:]
                if len(kept) != len(waits):
                    inst.sync_info = bass_rust.SyncInfo(
                        on_wait=kept, on_update=list(si.on_update))
                    changed = True
                out.append(inst)
            if changed or len(out) != len(insts):
                blk.instructions = out


def make_in_maps(x, W_attn, W_proj):
    x = np.ascontiguousarray(np.asarray(x, dtype=np.float32))
    W_attn = np.ascontiguousarray(np.asarray(W_attn, dtype=np.float32))
    W_proj = np.ascontiguousarray(np.asarray(W_proj, dtype=np.float32))
    # [k, q] layout: invalid where q < k gets -3e30 (becomes exp -> 0).
    # scaled so the exp's scale multiplier cancels.
    trimaskb = np.where(np.triu(np.ones((128, 128), bool)), 0.0, -3e30).astype(np.float32)
    ident = np.eye(128, dtype=np.float32)
    in_maps = []
    for c in range(N_CORES):
        b, hg = c // 4, c % 4
        cs = slice(hg * HG, (hg + 1) * HG)
        wa = np.concatenate(
            [W_attn[:, 0 * L:][:, cs], W_attn[:, 1 * L:][:, cs], W_attn[:, 2 * L:][:, cs]],
            axis=1,
        )
        import ml_dtypes
        bf16 = ml_dtypes.bfloat16
        in_maps.append({
            "xT": np.ascontiguousarray(x[b].T.astype(bf16)),
            "wa": np.ascontiguousarray(wa.astype(bf16)),
            "wp": np.ascontiguousarray(W_proj[cs, :].astype(bf16)),
            "trimaskb": trimaskb,
            "ident": ident,
        })
    return in_maps


_NC_CACHE = None


def kernel(x, W_attn, W_proj, **run_kwargs):
    global _NC_CACHE
    from concourse.bass_utils import run_bass_kernel_spmd

    if _NC_CACHE is None:
        _NC_CACHE = build_nc()
    nc = _NC_CACHE
    in_maps = make_in_maps(x, W_attn, W_proj)
    res = run_bass_kernel_spmd(nc, in_maps, list(range(N_CORES)), **run_kwargs)
    results = res.results if hasattr(res, "results") else res
    out = np.zeros((B, T, L), np.float32)
    for c in range(N_CORES):
        out[c // 4] += results[c]["out"]
    if run_kwargs:
        kernel.last_results = res
    return out


# revision 10
# speedup vs baseline: 1.3068x; 1.3068x over previous
"""Causal self-attention (B=2, T=2048, L=1024, H=16) on 8 TRN2 NeuronCores.

Sharding: tensor-parallel over heads (4 heads/core) x data-parallel over batch
(cores 0-3 -> batch 0, cores 4-7 -> batch 1). Each core computes its heads'
attention plus the partial output projection; the host sums the 4 partials
per batch.

Per-core pipeline (all matmul operands bf16):
  stage A: Q^T/K^T [256, 2048] (head dims on partitions), V [2048, 4*64].
  stage B per 512-wide q-block: score chunks S^T [128 k, <=512 q] computed as
    row-tiled matmul PAIRS (two heads' 64-wide contractions run concurrently
    in the PE array via tile_position rows 0-63 / 64-127). Chunks are scored
    in [128, 1024] double-bank PSUM tiles so exp runs over two chunks per
    activation call. exp (no max subtraction; scores ~N(0, 0.25)) -> bf16.
    PV accumulation as col-tiled matmul PAIRS (head A -> PSUM partitions
    0-63, head B -> 64-127). Softmax denominators accumulate in a shared
    PSUM bank via ones-weight matmuls col-tiled 4 ways (partition strips
    0/32/64/96).
  normalize: reciprocal of the four denominator rows, one broadcast matmul
    per head-pair expands 1/den to all 64 dv partitions, vector multiply.
  stage C: out[t, n] = y^T.T @ W_proj_slice accumulated in PSUM, DMA'd
    straight from PSUM to DRAM per [128, 512] tile.
"""

import sys

for _p in ("/opt/trn_rl_repo",):
    if _p not in sys.path:
        sys.path.insert(0, _p)

import numpy as np

import concourse.bass as bass
import concourse.mybir as mybir
import concourse.tile as tile

F32 = mybir.dt.float32
BF16 = mybir.dt.bfloat16
EXP = mybir.ActivationFunctionType.Exp

B, T, L = 2, 2048, 1024
H = 16
DH = 64                      # head dim
HPC = 4                      # heads per core
HG = HPC * DH                # 256 cols per core per q/k/v
N_CORES = 8
KC = T // 128                # 16 k-chunks
NQB = T // 512               # 4 q-blocks
SCALE = 1.0 / np.sqrt(np.float32(L))  # rsqrt(L) per reference


def build_nc(iters=1):
    nc = bass.Bass("TRN2", target_bir_lowering=False, debug=False)

    xT = nc.dram_tensor("xT", [L, T], BF16, kind="ExternalInput").ap()
    wa = nc.dram_tensor("wa", [L, 3 * HG], BF16, kind="ExternalInput").ap()
    wp = nc.dram_tensor("wp", [HG, L], BF16, kind="ExternalInput").ap()
    tm = nc.dram_tensor("trimaskb", [128, 128], F32, kind="ExternalInput").ap()
    idn = nc.dram_tensor("ident", [128, 128], F32, kind="ExternalInput").ap()
    out = nc.dram_tensor("out", [T, L], F32, kind="ExternalOutput").ap()

    with tile.TileContext(nc) as tc:
        with (
            tc.tile_pool(name="consts", bufs=1) as consts,
            tc.tile_pool(name="xp", bufs=8) as xp,
            tc.tile_pool(name="wap", bufs=8) as wap,
            tc.tile_pool(name="wpp", bufs=2) as wpp,
            tc.tile_pool(name="qk", bufs=2) as qk,
            tc.tile_pool(name="vp", bufs=16) as vp,
            tc.tile_pool(name="ytp", bufs=2) as ytp,
            tc.tile_pool(name="ptp", bufs=6) as ptp,
            tc.tile_pool(name="bcp", bufs=2) as bcp,
            tc.tile_pool(name="recp", bufs=4) as recp,
            tc.tile_pool(name="outp", bufs=2) as outp,
            tc.tile_pool(name="pss", bufs=4, space="PSUM") as pss,   # scores: 4 banks
            tc.tile_pool(name="pop", bufs=2, space="PSUM") as pop,   # po pair: 2 banks
            tc.tile_pool(name="p2", bufs=2, space="PSUM") as p2,     # bp/psmm/psc: 2 banks
        ):
            # ---- constants ----
            tm_sb = consts.tile([128, 128], BF16)
            nc.gpsimd.dma_start(out=tm_sb[:], in_=tm[:])
            id_sb = consts.tile([128, 128], BF16)
            nc.gpsimd.dma_start(out=id_sb[:], in_=idn[:])
            ones_sb = consts.tile([128, 128], BF16)
            nc.vector.memset(ones_sb[:], 1.0)
            for _it in range(iters):
                _body(nc, xT, wa, wp, out, tm_sb, id_sb, ones_sb,
                      xp, wap, wpp, qk, vp, ytp, ptp, bcp, recp, outp, pss, pop, p2, _it)

    import os as _os
    if not _os.environ.get("KERNEL_SKIP_WAITFIX"):
        _fix_matmul_waits(nc)
    return nc


def _body(nc, xT, wa, wp, out, tm_sb, id_sb, ones_sb,
          xp, wap, wpp, qk, vp, ytp, ptp, bcp, recp, outp, pss, pop, p2, it):
    # ---- input loads ----
    xt_sb = []
    wa_sb = []
    for kc in range(8):
        xt = xp.tile([128, T], BF16, tag="xt")
        nc.sync.dma_start(out=xt[:], in_=xT[kc * 128:(kc + 1) * 128, :])
        xt_sb.append(xt)
        wat = wap.tile([128, 3 * HG], BF16, tag="wa")
        nc.sync.dma_start(out=wat[:], in_=wa[kc * 128:(kc + 1) * 128, :])
        wa_sb.append(wat)
    wp_sb = []
    for i in range(2):
        wpt = wpp.tile([128, L], BF16, tag="wp")
        nc.sync.dma_start(out=wpt[:], in_=wp[i * 128:(i + 1) * 128, :])
        wp_sb.append(wpt)

    # ---- stage A: Q^T, K^T [256, T]; V [T, 4*64] ----
    qt = [qk.tile([128, T], BF16, tag="qt", name=f"qt{m}_{it}") for m in range(2)]
    kt = [qk.tile([128, T], BF16, tag="kt", name=f"kt{m}_{it}") for m in range(2)]
    for nb in range(NQB):
        for dst, coff, eng in ((qt, 0, "s"), (kt, HG, "v")):
            for m in range(2):
                ps = p2.tile([128, 512], F32, tag="p2")
                for kc in range(8):
                    nc.tensor.matmul(
                        ps[:],
                        wa_sb[kc][:, coff + m * 128:coff + (m + 1) * 128],
                        xt_sb[kc][:, nb * 512:(nb + 1) * 512],
                        start=(kc == 0),
                        stop=(kc == 7),
                    )
                d = dst[m][:, nb * 512:(nb + 1) * 512]
                if eng == "s":
                    nc.scalar.copy(d, ps[:])
                else:
                    nc.vector.tensor_copy(d, ps[:])

    va_sb = []
    for tt in range(KC):
        ps = p2.tile([128, 512], F32, tag="p2")
        for kc in range(8):
            nc.tensor.matmul(
                ps[:, 0:HG],
                xt_sb[kc][:, tt * 128:(tt + 1) * 128],
                wa_sb[kc][:, 2 * HG:3 * HG],
                start=(kc == 0),
                stop=(kc == 7),
            )
        va = vp.tile([128, HPC * 65], BF16, tag="va")
        nc.scalar.copy(
            va.rearrange("p (h c) -> p h c", c=65)[:, :, 0:64],
            ps[:, 0:HG].rearrange("p (h c) -> p h c", c=64)[:, :, :],
        )
        nc.vector.memset(va.rearrange("p (h c) -> p h c", c=65)[:, :, 64:65], 1.0)
        va_sb.append(va)

    # ---- stage B + C per q-block ----
    yt = [ytp.tile([128, T], BF16, tag="yt", name=f"yt{m}_{it}") for m in range(2)]
    for qb in range(NQB):
        nkc = 4 * qb + 4
        for pr in range(2):
            po = {}
            for hh in range(2):
                po[hh] = pop.tile([65, 512], F32, tag="po", name=f"po{qb}_{pr}_{hh}_{it}")

            def emit_av(kc, ncols, ptA, ptB):
                j = kc - 4 * qb
                a0 = 128 * j if j >= 0 else 0
                first, last = kc == 0, kc == nkc - 1
                mid = not (first or last)
                for hh, pt in ((0, ptA), (1, ptB)):
                    h = 2 * pr + hh
                    nc.tensor.matmul(
                        po[hh][:, a0:512],
                        va_sb[kc][:, h * 65:(h + 1) * 65],
                        pt[:, 0:ncols],
                        start=first, stop=last, skip_group_check=mid,
                    )

            prev = None
            for kc in range(nkc):
                j = kc - 4 * qb
                q0 = qb * 512 + (128 * j if j >= 0 else 0)
                ncols = 512 - (128 * j if j >= 0 else 0)
                psA = pss.tile([128, 512], F32, tag="ps2")
                psB = pss.tile([128, 512], F32, tag="ps2")
                for hh, ps2 in ((0, psA), (1, psB)):
                    # two heads' 64-contract scores run concurrently via
                    # row tiling (partitions 0-63 / 64-127)
                    nc.tensor.matmul(
                        ps2[:, 0:ncols],
                        kt[pr][hh * 64:(hh + 1) * 64, kc * 128:(kc + 1) * 128],
                        qt[pr][hh * 64:(hh + 1) * 64, q0:q0 + ncols],
                        start=True,
                        stop=(j < 0),
                    )
                    if j >= 0:
                        # -3e30 above the diagonal: psum += I.T @ tri_bias
                        nc.tensor.matmul(
                            ps2[:, 0:128],
                            id_sb[:],
                            tm_sb[:],
                            start=False,
                            stop=True,
                        )
                if prev is not None:
                    emit_av(*prev)
                ptA = ptp.tile([128, 512], BF16, tag="pt")
                ptB = ptp.tile([128, 512], BF16, tag="pt")
                nc.scalar.activation(ptA[:, 0:ncols], psA[:, 0:ncols], EXP,
                                     scale=float(SCALE))
                nc.scalar.activation(ptB[:, 0:ncols], psB[:, 0:ncols], EXP,
                                     scale=float(SCALE))
                prev = (kc, ncols, ptA, ptB)
            emit_av(*prev)

            # ---- normalize: yT = po[0:64] * broadcast(1/po[64]) ----
            bs = bcp.tile([128, 512], F32, tag="bs")
            for hh in range(2):
                rec = recp.tile([65, 512], BF16, tag="rec")
                with nc.allow_low_precision(reason="softmax denom recip to bf16"):
                    nc.vector.reciprocal(rec[64:65, :], po[hh][64:65, :])
                bp = p2.tile([128, 512], F32, tag="p2")
                nc.tensor.matmul(bp[:], ones_sb[64:65, :],
                                 rec[64:65, :], start=True, stop=True)
                nc.vector.tensor_copy(bs[hh * 64:(hh + 1) * 64, :],
                                      bp[hh * 64:(hh + 1) * 64, :])
            for hh in range(2):
                nc.vector.tensor_mul(
                    yt[pr][hh * 64:(hh + 1) * 64, qb * 512:(qb + 1) * 512],
                    po[hh][0:64, :],
                    bs[hh * 64:(hh + 1) * 64, :],
                )

        # ---- stage C for this q-block's 4 t-tiles ----
        for tt in range(4 * qb, 4 * qb + 4):
            osb = outp.tile([128, L], F32, tag="osb")
            for nn in range(2):
                ps = p2.tile([128, 512], F32, tag="p2")
                for pr in range(2):
                    nc.tensor.matmul(
                        ps[:],
                        yt[pr][:, tt * 128:(tt + 1) * 128],
                        wp_sb[pr][:, nn * 512:(nn + 1) * 512],
                        start=(pr == 0),
                        stop=(pr == 1),
                    )
                nc.vector.tensor_copy(osb[:, nn * 512:(nn + 1) * 512], ps[:])
            nc.sync.dma_start(out=out[tt * 128:(tt + 1) * 128, :], in_=osb[:])


def _fix_matmul_waits(nc):
    """walrus caps sync-wait commands at one per hardware instruction.
    Tile can emit more. Two safe fixes, applied in order:
    1. drop waits on the instruction's own engine semaphore that are already
       guaranteed by in-order retirement of earlier same-stream instructions;
    2. for any instruction still holding >1 wait, insert same-engine NoOps
       immediately before it, each carrying one excess wait (the waits still
       all execute before the instruction dispatches).
    """
    import bass_rust
    import concourse.mybir as mybir

    SKIP = (mybir.InstEventSemaphore, mybir.InstCall,
            mybir.InstUnconditionalBranch)
    nop_id = [0]

    for f in nc.m.functions:
        for blk in f.blocks:
            insts = list(blk.instructions)
            out = []
            changed = False
            for i, inst in enumerate(insts):
                si = inst.sync_info
                eng = getattr(inst, "engine", None)
                if si is None or eng is None or isinstance(inst, SKIP):
                    out.append(inst)
                    continue
                waits = list(si.on_wait)
                kept = waits
                if len(kept) > 1:
                    for w in kept[:-1]:
                        nop = mybir.InstNoOp(name=f"I-waitnop-{nop_id[0]}")
                        nop_id[0] += 1
                        nop.engine = eng
                        nop.sync_info = bass_rust.SyncInfo(on_wait=[w], on_update=[])
                        out.append(nop)
                    kept = kept[-1:]
                if len(kept) != len(waits):
                    inst.sync_info = bass_rust.SyncInfo(
                        on_wait=kept, on_update=list(si.on_update))
                    changed = True
                out.append(inst)
            if changed or len(out) != len(insts):
                blk.instructions = out


def make_in_maps(x, W_attn, W_proj):
    x = np.ascontiguousarray(np.asarray(x, dtype=np.float32))
    W_attn = np.ascontiguousarray(np.asarray(W_attn, dtype=np.float32))
    W_proj = np.ascontiguousarray(np.asarray(W_proj, dtype=np.float32))
    # [k, q] layout: invalid where q < k gets -3e30 (becomes exp -> 0).
    # scaled so the exp's scale multiplier cancels.
    trimaskb = np.where(np.triu(np.ones((128, 128), bool)), 0.0, -3e30).astype(np.float32)
    ident = np.eye(128, dtype=np.float32)
    in_maps = []
    for c in range(N_CORES):
        b, hg = c // 4, c % 4
        cs = slice(hg * HG, (hg + 1) * HG)
        wa = np.concatenate(
            [W_attn[:, 0 * L:][:, cs], W_attn[:, 1 * L:][:, cs], W_attn[:, 2 * L:][:, cs]],
            axis=1,
        )
        import ml_dtypes
        bf16 = ml_dtypes.bfloat16
        in_maps.append({
            "xT": np.ascontiguousarray(x[b].T.astype(bf16)),
            "wa": np.ascontiguousarray(wa.astype(bf16)),
            "wp": np.ascontiguousarray(W_proj[cs, :].astype(bf16)),
            "trimaskb": trimaskb,
            "ident": ident,
        })
    return in_maps


_NC_CACHE = None


def kernel(x, W_attn, W_proj, **run_kwargs):
    global _NC_CACHE
    from concourse.bass_utils import run_bass_kernel_spmd

    if _NC_CACHE is None:
        _NC_CACHE = build_nc()
    nc = _NC_CACHE
    in_maps = make_in_maps(x, W_attn, W_proj)
    res = run_bass_kernel_spmd(nc, in_maps, list(range(N_CORES)), **run_kwargs)
    results = res.results if hasattr(res, "results") else res
    out = np.zeros((B, T, L), np.float32)
    for c in range(N_CORES):
        out[c // 4] += results[c]["out"]
    if run_kwargs:
        kernel.last_results = res
    return out
